# revision 4
# baseline (speedup 1.0000x reference)
"""Trainium2 Bass kernel for nn_BoundaryAwareLoss (8 NeuronCores).

Sharding: B*H = 2*512 = 1024 rows split into 8 slabs of 128 rows; core c
handles batch b = c//4, rows [128*(c%4), 128*(c%4)+128).  All per-pixel work
(sobel boundary, weighted CE, embedding segment sums, hinge distances) runs
on device; the host only reshapes/casts inputs, all-reduces the tiny per-core
partials (K=16 x D=32 segment sums, per-partition scalars) between the two
launches, and assembles the final 4 scalars (incl. the K x K center-pair term,
which is O(K^2*D) ~ 8K flops).

Launch 1: sobel boundary weights + weighted cross-entropy partials + per-(k)
          embedding segment sums (1024 accumulating PE matmuls of
          onehot[128,16]^T @ emb[128,32]).
Launch 2: per-pixel center gather (gpsimd indirect_copy), ||e-c||^2, sqrt,
          hinge, per-k hinge segment sums via fused tensor_tensor_reduce.
"""

import sys

if "/opt/trn_rl_repo" not in sys.path:
    sys.path.insert(0, "/opt/trn_rl_repo")

from contextlib import ExitStack

import ml_dtypes
import numpy as np

import concourse.bass as bass
import concourse.tile as tile
from concourse import bacc, mybir
from concourse.bass_utils import run_bass_kernel_spmd

BF16 = mybir.dt.bfloat16
F32 = mybir.dt.float32
U16 = mybir.dt.uint16

NUM_CLASSES = 19
K = 16
D = 32
B, H, W = 2, 512, 1024
ROWS = 128          # rows per core
NPIX = ROWS * W     # pixels per core
DELTA_V = 0.5
DELTA_D = 1.5

_cache = {}
last_results = []  # debug: populated with BassKernelResults on each call


def _build_launch1():
    nc = bacc.Bacc("TRN2", target_bir_lowering=False, debug=False, num_devices=8)
    sem_t = nc.dram_tensor("sem_t", [ROWS, W * NUM_CLASSES], BF16, kind="ExternalInput").ap()
    lab3 = nc.dram_tensor("lab3", [3, ROWS, W + 2], F32, kind="ExternalInput").ap()
    ilab = nc.dram_tensor("ilab", [ROWS, W], F32, kind="ExternalInput").ap()
    inst_t = nc.dram_tensor("inst_t", [ROWS, W * D], BF16, kind="ExternalInput").ap()
    o_ce = nc.dram_tensor("o_ce", [ROWS, 1], F32, kind="ExternalOutput").ap()
    o_w = nc.dram_tensor("o_w", [ROWS, 1], F32, kind="ExternalOutput").ap()
    o_sums = nc.dram_tensor("o_sums", [K, D], F32, kind="ExternalOutput").ap()

    with tile.TileContext(nc) as tc, ExitStack() as ctx:
        big = ctx.enter_context(tc.tile_pool(name="big", bufs=1))
        sml = ctx.enter_context(tc.tile_pool(name="sml", bufs=1))
        pp = ctx.enter_context(tc.tile_pool(name="pp", bufs=1, space="PSUM"))

        t_sem = big.tile([ROWS, W * NUM_CLASSES], BF16, tag="sem")
        nc.sync.dma_start(t_sem[:], sem_t[:])
        t_inst = big.tile([ROWS, W * D], BF16, tag="inst")
        nc.sync.dma_start(t_inst[:], inst_t[:])
        t_l3 = [sml.tile([ROWS, W + 2], F32, tag=f"l3_{i}", name=f"l3_{i}") for i in range(3)]
        for i in range(3):
            nc.sync.dma_start(t_l3[i][:], lab3[i])
        t_il = sml.tile([ROWS, W], F32, tag="il")
        nc.sync.dma_start(t_il[:], ilab[:])

        # ---- sobel boundary weights (zero-padded 3x3, labels as float) ----
        lm, l0, lp = t_l3
        dh = [sml.tile([ROWS, W], F32, tag=f"dh{i}", name=f"dh{i}") for i in range(3)]
        for i, t in enumerate(t_l3):
            nc.vector.tensor_sub(dh[i][:], t[:, 2:], t[:, :-2])
        gx = sml.tile([ROWS, W], F32, tag="gx")
        nc.vector.tensor_add(gx[:], dh[0][:], dh[2][:])
        nc.vector.tensor_scalar_mul(dh[1][:], dh[1][:], 2.0)
        nc.vector.tensor_add(gx[:], gx[:], dh[1][:])
        # smooth_h of row-neighbours for gy
        sh = [sml.tile([ROWS, W], F32, tag=f"sh{i}", name=f"sh{i}") for i in range(2)]
        for i, t in enumerate((lm, lp)):
            nc.vector.tensor_add(sh[i][:], t[:, 2:], t[:, :-2])
            tmp = sml.tile([ROWS, W], F32, tag="shtmp")
            nc.vector.tensor_scalar_mul(tmp[:], t[:, 1:-1], 2.0)
            nc.vector.tensor_add(sh[i][:], sh[i][:], tmp[:])
        gy = sml.tile([ROWS, W], F32, tag="gy")
        nc.vector.tensor_sub(gy[:], sh[1][:], sh[0][:])
        nc.vector.tensor_mul(gx[:], gx[:], gx[:])
        nc.vector.tensor_mul(gy[:], gy[:], gy[:])
        nc.vector.tensor_add(gx[:], gx[:], gy[:])  # gx = mag^2
        wts = sml.tile([ROWS, W], F32, tag="wts")
        # boundary if mag^2 > 0.01  (labels < 255 always here, so valid == 1)
        nc.vector.tensor_scalar(wts[:], gx[:], 0.01, None, op0=mybir.AluOpType.is_gt)
        nc.vector.tensor_scalar_add(wts[:], wts[:], 1.0)

        # ---- CE: gather x_label, then in-place exp + per-pixel sumexp ----
        gath = sml.tile([ROWS, W], F32, tag="gath")
        sem3 = t_sem[:].rearrange("p (w c) -> p w c", c=NUM_CLASSES)
        mask = sml.tile([ROWS, W], mybir.dt.uint8, tag="mask")
        nc.vector.tensor_copy(gath[:], sem3[:, :, 0])
        for c in range(1, NUM_CLASSES):
            nc.vector.tensor_scalar(mask[:], l0[:, 1:-1], float(c), None,
                                    op0=mybir.AluOpType.is_equal)
            nc.vector.copy_predicated(gath[:], mask[:], sem3[:, :, c])
        nc.scalar.activation(t_sem[:], t_sem[:], mybir.ActivationFunctionType.Exp)
        sume = sml.tile([ROWS, W], F32, tag="sume")
        nc.vector.reduce_sum(sume[:], sem3, axis=mybir.AxisListType.X)
        logz = sml.tile([ROWS, W], F32, tag="logz")
        nc.scalar.activation(logz[:], sume[:], mybir.ActivationFunctionType.Ln)
        nll = sml.tile([ROWS, W], F32, tag="nll")
        nc.vector.tensor_sub(nll[:], logz[:], gath[:])
        nc.vector.tensor_mul(nll[:], nll[:], wts[:])
        ce_p = sml.tile([ROWS, 1], F32, tag="cep")
        nc.vector.reduce_sum(ce_p[:], nll[:], axis=mybir.AxisListType.X)
        w_p = sml.tile([ROWS, 1], F32, tag="wp")
        nc.vector.reduce_sum(w_p[:], wts[:], axis=mybir.AxisListType.X)
        nc.sync.dma_start(o_ce[:], ce_p[:])
        nc.sync.dma_start(o_w[:], w_p[:])

        # ---- instance segment sums: onehot build + accumulating PE matmuls ----
        oh = big.tile([ROWS, W * K], BF16, tag="oh")
        oh3 = oh[:].rearrange("p (w k) -> p w k", k=K)
        for k in range(K):
            nc.vector.tensor_scalar(oh3[:, :, k], t_il[:], float(k), None,
                                    op0=mybir.AluOpType.is_equal)
        ps = pp.tile([K, D], F32, tag="ps")
        inst3 = t_inst[:].rearrange("p (w d) -> p w d", d=D)
        for j in range(W):
            nc.tensor.matmul(ps[:], oh3[:, j, :], inst3[:, j, :],
                             start=(j == 0), stop=(j == W - 1))
        sums_sb = sml.tile([K, D], F32, tag="sums_sb")
        nc.vector.tensor_copy(sums_sb[:], ps[:])
        nc.sync.dma_start(o_sums[:], sums_sb[:])
    nc.compile()
    return nc


def _build_launch2():
    import os
    STAGE = int(os.environ.get("LAUNCH2_STAGE", "4"))
    nc = bacc.Bacc("TRN2", target_bir_lowering=False, debug=False, num_devices=8)
    inst_t = nc.dram_tensor("inst_t", [ROWS, W * D], BF16, kind="ExternalInput").ap()
    ilab = nc.dram_tensor("ilab", [ROWS, W], F32, kind="ExternalInput").ap()
    cent = nc.dram_tensor("cent", [ROWS, K * D], BF16, kind="ExternalInput").ap()
    o_hs = nc.dram_tensor("o_hs", [ROWS, K], F32, kind="ExternalOutput").ap()

    with tile.TileContext(nc) as tc, ExitStack() as ctx:
        big = ctx.enter_context(tc.tile_pool(name="big", bufs=1))
        sml = ctx.enter_context(tc.tile_pool(name="sml", bufs=1))

        t_inst = big.tile([ROWS, W * D], BF16, tag="inst")
        nc.sync.dma_start(t_inst[:], inst_t[:])
        t_cent = sml.tile([ROWS, K * D], BF16, tag="cent")
        nc.sync.dma_start(t_cent[:], cent[:])
        t_il = sml.tile([ROWS, W], F32, tag="il")
        nc.sync.dma_start(t_il[:], ilab[:])

        m16 = sml.tile([ROWS, K * W], mybir.dt.uint8, tag="m16")
        for k in range(K):
            nc.vector.tensor_scalar(m16[:, k * W:(k + 1) * W], t_il[:], float(k),
                                    None, op0=mybir.AluOpType.is_equal)
        hs = sml.tile([ROWS, K], F32, tag="hs")
        dist = sml.tile([ROWS, W], F32, tag="dist")

        cg = big.tile([ROWS, W * D], BF16, tag="cg")
        cg3 = cg[:].rearrange("p (w d) -> p w d", d=D)
        if STAGE >= 2:
            # cg[p, w, :] = cent[p, ilab[p, w], :]  (16 masked predicated copies,
            # innermost dim contiguous on out/data; mask broadcast over D)
            for k in range(K):
                mask_v = m16[:, k * W:(k + 1) * W].broadcast_to([ROWS, W, D])
                data_v = t_cent[:, k * D:(k + 1) * D].broadcast_to(
                    [ROWS, D, W]).rearrange("p d w -> p w d")
                if k == 0:
                    nc.vector.tensor_copy(cg3, data_v)
                else:
                    nc.vector.copy_predicated(cg3, mask_v, data_v)
        if STAGE >= 3:
            nc.vector.tensor_sub(cg[:], t_inst[:], cg[:])
            nc.vector.tensor_mul(cg[:], cg[:], cg[:])
            d2 = sml.tile([ROWS, W], F32, tag="d2")
            nc.vector.reduce_sum(d2[:], cg3, axis=mybir.AxisListType.X)
            nc.scalar.activation(dist[:], d2[:], mybir.ActivationFunctionType.Sqrt)
            nc.vector.tensor_scalar_add(dist[:], dist[:], -DELTA_V)
            nc.vector.tensor_scalar_max(dist[:], dist[:], 0.0)
            nc.vector.tensor_mul(dist[:], dist[:], dist[:])
        else:
            nc.vector.tensor_copy(dist[:], t_il[:])
        if STAGE >= 4:
            scr = sml.tile([ROWS, W], F32, tag="scr")
            for k in range(K):
                nc.vector.tensor_mul(scr[:], dist[:], m16[:, k * W:(k + 1) * W])
                nc.vector.reduce_sum(hs[:, k:k + 1], scr[:],
                                     axis=mybir.AxisListType.X)
        else:
            nc.gpsimd.memset(hs[:], 0.0)
        nc.sync.dma_start(o_hs[:], hs[:])
    nc.compile()
    return nc


def _get_programs():
    if "l1" not in _cache:
        _cache["l1"] = _build_launch1()
        _cache["l2"] = _build_launch2()
    return _cache["l1"], _cache["l2"]


def kernel(semantic_logits, instance_logits, semantic_labels, instance_labels,
           _return_time=False):
    nc1, nc2 = _get_programs()
    bf16 = ml_dtypes.bfloat16
    cores = list(range(8))

    # padded label canvas per batch for the 3 row-shifted sobel tiles
    lab_pad = np.zeros((B, H + 2, W + 2), np.float32)
    lab_pad[:, 1:-1, 1:-1] = semantic_labels.astype(np.float32)

    in1, in2_base = [], []
    for c in cores:
        b, r0 = c // 4, 128 * (c % 4)
        sem = semantic_logits[b, :, r0:r0 + ROWS, :].transpose(1, 2, 0)
        inst = instance_logits[b, :, r0:r0 + ROWS, :].transpose(1, 2, 0)
        il = instance_labels[b, r0:r0 + ROWS, :]
        lab3 = np.stack([lab_pad[b, r0 + i:r0 + i + ROWS] for i in range(3)])
        in1.append({
            "sem_t": np.ascontiguousarray(sem).reshape(ROWS, -1).astype(bf16),
            "lab3": np.ascontiguousarray(lab3),
            "ilab": il.astype(np.float32),
            "inst_t": np.ascontiguousarray(inst).reshape(ROWS, -1).astype(bf16),
        })
        in2_base.append({
            "inst_t": in1[-1]["inst_t"],
            "ilab": in1[-1]["ilab"],
        })

    r1 = run_bass_kernel_spmd(nc1, in1, core_ids=cores)
    last_results.clear()
    last_results.append(r1)

    # host: combine tiny partials -> centers
    counts = np.stack([np.bincount(instance_labels[b].ravel(), minlength=K)
                       for b in range(B)]).astype(np.float32)
    sums = np.zeros((B, K, D), np.float32)
    ce_num = 0.0
    w_sum = 0.0
    for c in cores:
        sums[c // 4] += r1.results[c]["o_sums"]
        ce_num += float(r1.results[c]["o_ce"].sum())
        w_sum += float(r1.results[c]["o_w"].sum())
    centers = sums / np.maximum(counts, 1.0)[:, :, None]

    in2 = []
    for c in cores:
        m = dict(in2_base[c])
        m["cent"] = np.broadcast_to(
            centers[c // 4].astype(bf16)[None], (ROWS, K, D)).reshape(ROWS, -1).copy()
        in2.append(m)
    r2 = run_bass_kernel_spmd(nc2, in2, core_ids=cores)
    last_results.append(r2)

    hsum = np.zeros((B, K), np.float32)
    for c in cores:
        hsum[c // 4] += r2.results[c]["o_hs"].sum(axis=0)

    # final scalar assembly (identical math to the reference)
    present = (counts > 0) & (np.arange(K)[None, :] != 0)
    var_k = hsum / np.maximum(counts, 1.0) * present
    n_var = present.sum()
    loss_var = var_k.sum() / max(n_var, 1.0)
    loss_dist_n, n_dist = 0.0, 0
    for b in range(B):
        cd = centers[b][:, None, :] - centers[b][None, :, :]
        sq = (cd * cd).sum(-1)
        pair = present[b][:, None] & present[b][None, :] & ~np.eye(K, dtype=bool)
        pd = np.sqrt(np.where(pair, sq, 1.0))
        dh = np.square(np.maximum(2.0 * DELTA_D - pd, 0.0)) * pair
        n_pairs = pair.sum()
        dl = dh.sum() / max(n_pairs, 1.0)
        if present[b].sum() > 1:
            loss_dist_n += dl
            n_dist += 1
    loss_dist = loss_dist_n / max(n_dist, 1)
    instance_loss = loss_var + loss_dist
    semantic_loss = ce_num / (w_sum + 1e-8)
    mean_pw = w_sum / (B * H * W)
    total = semantic_loss + instance_loss
    out = np.array([total, semantic_loss, instance_loss, mean_pw], np.float32)
    if _return_time:
        return out, (r1.exec_time_ns, r2.exec_time_ns)
    return out

